# revision 21
# baseline (speedup 1.0000x reference)
"""GAT layer (B=8, N=2048, F=64) on 8 trn2 NeuronCores.

Strategy: data-parallel over batch B — one graph per core, adj replicated.

Math: with e = leaky_relu(e1_i + e2_j), exp(e - 0.2*e1_i) (row factor
cancels in softmax) = A2_j * max(G_i, r_j) where G = exp(0.8*e1),
A2 = exp(e2), r = exp(-0.8*e2). A2 folds into the matmul weights on the
host (whA = [Wh*A2 | A2]; row 64 yields softmax denominators), so the
device computes q_ij = max(G_i, r_j) * adj_ji and accumulates
outT[65, i] += whA_t^T @ q_t over 16 j-tiles. Divide + elu epilogue is
O(N*F) and runs on the host.

The N^2 masking never touches a compute engine: adj is encoded on the
host as {0, -448} (exact in fp8e4) and ADDED into the score tiles by
accumulating DMAs (gpsimd software-DGE queue, the only one that can
cast+accumulate; <=2048 cols per DMA, the hardware limit). The mask
then falls out of a relu, and BOTH remaining N^2 ops run in DVE's 4x
tensor_scalar mode (~0.65us/tile each):
    s_t = (G max r_t)          # DVE 4x
    s_t += adjenc_t            # DMA engines (RMW), fp8 stream from HBM
    q_t = relu(s_t)            # DVE 4x, merged over 2-tile stages
DVE total ~20us; adj HBM traffic halves (fp8); SBUF never stores adj.
The two-op DVE alternative (mask via tensor_tensor, 2x mode) costs
~30us and fp16 adj; fused scalar_tensor_tensor is 1x (~37us); Pool
tensor ops poison DVE's fast modes (all measured).

Startup/tail: G is host-replicated and DMA'd in 4 chunks on 4 queues
(single-queue DMA ~44GB/s, stride-0 broadcast much worse); junk
matmuls ramp PE's clock (2.4GHz needs ~3us continuous busy) before the
real stream; the 16-tile accumulation runs as two 4-bank PSUM phases
whose copies/output DMAs overlap the other phase; host adds the halves.
"""

import sys

import numpy as np
import ml_dtypes

for _p in ("/opt/trn_rl_repo",):
    if _p not in sys.path:
        sys.path.insert(0, _p)

from contextlib import ExitStack

import concourse.bass as bass
import concourse.tile as tile
from concourse import bacc, mybir
from concourse.bass_utils import run_bass_kernel_spmd

B, N, F = 8, 2048, 64
P = 128
T = N // P  # 16 j-tiles
NB = N // 512  # 4 psum banks of moving-free 512
STAGE = 2  # j-tiles per s/q stage (one merged relu per stage)
NWARM = 6  # PE clock-ramp matmuls during startup

_CACHE = {}


def _build_program():
    if "nc" in _CACHE:
        return _CACHE["nc"]
    dt = mybir.dt
    nc = bacc.Bacc("TRN2", target_bir_lowering=False, debug=False)

    adjenc = nc.dram_tensor(
        "adjenc", [P, T * N], dt.float8e4, kind="ExternalInput"
    ).ap()
    g = nc.dram_tensor("g", [P, N], dt.float16, kind="ExternalInput").ap()
    rsc = nc.dram_tensor("rsc", [P, T], dt.float32, kind="ExternalInput").ap()
    wha = nc.dram_tensor("wha", [P, T * 65], dt.float16, kind="ExternalInput").ap()
    outA = nc.dram_tensor("outA", [65, N], dt.float16, kind="ExternalOutput").ap()
    outB = nc.dram_tensor("outB", [65, N], dt.float16, kind="ExternalOutput").ap()

    with tile.TileContext(nc) as tc, ExitStack() as ctx:
        singles = ctx.enter_context(tc.tile_pool(name="singles", bufs=1))
        spool = ctx.enter_context(tc.tile_pool(name="spool", bufs=3))
        qpool = ctx.enter_context(tc.tile_pool(name="qpool", bufs=3))
        accp = ctx.enter_context(tc.tile_pool(name="accp", bufs=1, space="PSUM"))

        rsc_sb = singles.tile([P, T], dt.float32)
        nc.scalar.dma_start(out=rsc_sb[:], in_=rsc)
        g_sb = singles.tile([P, N], dt.float16)
        for qi, eng in enumerate((nc.sync, nc.scalar, nc.gpsimd, nc.sync)):
            eng.dma_start(
                out=g_sb[:, qi * 512 : (qi + 1) * 512],
                in_=g[:, qi * 512 : (qi + 1) * 512],
            )
        wha_sb = singles.tile([P, T * 65], dt.float16)
        nc.scalar.dma_start(out=wha_sb[:], in_=wha)

        accs = {}
        for ph in range(2):
            for n in range(NB):
                accs[ph, n] = accp.tile(
                    [65, 512], dt.float32, tag=f"acc{ph}_{n}", name=f"acc{ph}_{n}"
                )

        # PE clock-ramp: junk matmuls into the (not yet live) phase-B banks.
        for w in range(NWARM):
            nc.tensor.matmul(
                out=accs[1, w % NB][:],
                lhsT=wha_sb[:, 0:65],
                rhs=g_sb[:, 0:512],
                start=True,
                stop=True,
            )

        osbA = singles.tile([65, N], dt.float16, name="osbA")
        osbB = singles.tile([65, N], dt.float16, name="osbB")

        for st in range(T // STAGE):
            s = spool.tile([P, STAGE * N], dt.float16, name="s")
            q = qpool.tile([P, STAGE * N], dt.float16, name="q")
            for k in range(STAGE):
                t = st * STAGE + k
                nc.vector.tensor_scalar_max(
                    s[:, k * N : (k + 1) * N], g_sb[:], rsc_sb[:, t : t + 1]
                )
            for k in range(STAGE):
                t = st * STAGE + k
                nc.gpsimd.dma_start(
                    out=s[:, k * N : (k + 1) * N],
                    in_=adjenc[:, t * N : (t + 1) * N],
                    accum_op=mybir.AluOpType.add,
                )
            nc.vector.tensor_scalar_max(q[:], s[:], 0.0)
            for k in range(STAGE):
                t = st * STAGE + k
                ph, first, last = t // 8, t % 8 == 0, t % 8 == 7
                for n in range(NB):
                    nc.tensor.matmul(
                        out=accs[ph, n][:],
                        lhsT=wha_sb[:, t * 65 : (t + 1) * 65],
                        rhs=q[:, k * N + n * 512 : k * N + (n + 1) * 512],
                        start=first,
                        stop=last,
                    )
                if last:
                    osb, outd = (osbA, outA) if ph == 0 else (osbB, outB)
                    for n in range(NB):
                        dst = osb[:, n * 512 : (n + 1) * 512]
                        if n % 2 == 0:
                            nc.scalar.copy(dst, accs[ph, n][:])
                        else:
                            nc.vector.tensor_copy(dst, accs[ph, n][:])
                    nc.sync.dma_start(out=outd[:, 0 : N // 2], in_=osb[:, 0 : N // 2])
                    nc.sync.dma_start(out=outd[:, N // 2 :], in_=osb[:, N // 2 :])

    nc.compile()
    _CACHE["nc"] = nc
    return nc


def _prep_inputs(h, adj, W, a):
    h = np.asarray(h, np.float32)
    adj = np.asarray(adj, np.float32)
    W = np.asarray(W, np.float32)
    a = np.asarray(a, np.float32)

    # adj^T tiles side by side, encoded {edge: 0, no edge: -448}.
    adjenc = np.ascontiguousarray(
        ((adj.T - 1.0) * 448.0).reshape(T, P, N).transpose(1, 0, 2).reshape(P, T * N)
    ).astype(ml_dtypes.float8_e4m3)

    Wh = np.einsum("bnf,of->bno", h, W)  # [B, N, F]
    e1 = Wh @ a[:F]  # [B, N]
    e2 = Wh @ a[F:]  # [B, N]
    A2 = np.exp(e2)
    G = np.exp(0.8 * e1).astype(np.float16)  # [B, N]
    r = np.exp(-0.8 * e2).astype(np.float32)  # [B, N]
    whA = np.concatenate([Wh * A2[..., None], A2[..., None]], axis=2)  # [B, N, 65]
    whA = np.ascontiguousarray(
        whA.reshape(B, T, P, 65).transpose(0, 2, 1, 3)
    ).reshape(B, P, T * 65)

    in_maps = []
    for b in range(B):
        in_maps.append(
            {
                "adjenc": adjenc,
                "g": np.ascontiguousarray(np.broadcast_to(G[b], (P, N))),
                "rsc": np.ascontiguousarray(r[b].reshape(T, P).T),
                "wha": whA[b].astype(np.float16),
            }
        )
    return in_maps


def kernel(h, adj, W, a, _trace=False):
    nc = _build_program()
    in_maps = _prep_inputs(h, adj, W, a)
    res = run_bass_kernel_spmd(nc, in_maps, list(range(B)), trace=_trace)
    outs = np.empty((B, N, F), np.float32)
    for b in range(B):
        outT = np.asarray(res.results[b]["outA"], np.float32) + np.asarray(
            res.results[b]["outB"], np.float32
        )
        hp = outT[:F].T / outT[F][:, None]
        outs[b] = np.where(hp > 0, hp, np.expm1(hp))
    if _trace:
        kernel.last_results = res
    return outs


# revision 22
# speedup vs baseline: 1.0033x; 1.0033x over previous
"""GAT layer (B=8, N=2048, F=64) on 8 trn2 NeuronCores.

Strategy: data-parallel over batch B — one graph per core, adj replicated.

Math: with e = leaky_relu(e1_i + e2_j), exp(e - 0.2*e1_i) (row factor
cancels in softmax) = A2_j * max(G_i, r_j) where G = exp(0.8*e1),
A2 = exp(e2), r = exp(-0.8*e2). A2 folds into the matmul weights on the
host (whA = [Wh*A2 | A2]; row 64 yields softmax denominators), so the
device computes q_ij = max(G_i, r_j) * adj_ji and accumulates
outT[65, i] += whA_t^T @ q_t over 16 j-tiles. Divide + elu epilogue is
O(N*F) and runs on the host.

The N^2 stage splits across two masking paths, tuned from HW traces:
  - 4 j-tiles (first in the PE chain): adj fp16 in SBUF, DVE
    tensor_tensor mask in 2x mode (~1.2us) — available earliest.
  - 12 j-tiles: adj encoded {0,-448} fp8 and ADDED into score tiles by
    accumulating DMAs (gpsimd software-DGE, the only queue that can
    cast+accumulate; one <=2048-col DMA per tile, ~1.1us issue each);
    the mask then falls out of relu = tensor_scalar_max(s, 0), which
    runs in DVE 4x mode and is merged over 4-tile groups (~0.56us/tile).
  All scores s_t = (G max r_t) run in DVE 4x mode (~0.65us).
  DVE busy ~22us; the accum-DMA issue chain ~15us hides under it.
  (Alternatives measured: all-tensor_tensor ~30us DVE; fused
  scalar_tensor_tensor 1x ~37us; Pool tensor ops poison DVE fast modes.)

Startup/tail: G is host-replicated and DMA'd in 3 chunks on 3 queues
(single-queue DMA is ~44GB/s); junk matmuls ramp PE's clock during the
DMA fill; the 16-tile accumulation runs as two 4-bank PSUM phases whose
copies (split ACT/DVE) + halved output DMAs overlap the other phase;
host adds the halves.
"""

import sys

import numpy as np
import ml_dtypes

for _p in ("/opt/trn_rl_repo",):
    if _p not in sys.path:
        sys.path.insert(0, _p)

from contextlib import ExitStack

import concourse.bass as bass
import concourse.tile as tile
from concourse import bacc, mybir
from concourse.bass_utils import run_bass_kernel_spmd

B, N, F = 8, 2048, 64
P = 128
T = N // P  # 16 j-tiles
NB = N // 512  # 4 psum banks of moving-free 512
NTT = 4  # leading tiles masked via DVE tensor_tensor (fp16 adj)
GRP = 4  # accum-DMA tiles per merged relu group
NWARM = 6  # PE clock-ramp matmuls during startup

_CACHE = {}


def _build_program():
    if "nc" in _CACHE:
        return _CACHE["nc"]
    dt = mybir.dt
    nc = bacc.Bacc("TRN2", target_bir_lowering=False, debug=False)

    adj16 = nc.dram_tensor("adj16", [P, NTT * N], dt.float16, kind="ExternalInput").ap()
    adjenc = nc.dram_tensor(
        "adjenc", [P, (T - NTT) * N], dt.float8e4, kind="ExternalInput"
    ).ap()
    g = nc.dram_tensor("g", [P, N], dt.float16, kind="ExternalInput").ap()
    rsc = nc.dram_tensor("rsc", [P, T], dt.float32, kind="ExternalInput").ap()
    wha = nc.dram_tensor("wha", [P, T * 65], dt.float16, kind="ExternalInput").ap()
    outA = nc.dram_tensor("outA", [65, N], dt.float16, kind="ExternalOutput").ap()
    outB = nc.dram_tensor("outB", [65, N], dt.float16, kind="ExternalOutput").ap()

    with tile.TileContext(nc) as tc, ExitStack() as ctx:
        singles = ctx.enter_context(tc.tile_pool(name="singles", bufs=1))
        qpool = ctx.enter_context(tc.tile_pool(name="qpool", bufs=3))
        accp = ctx.enter_context(tc.tile_pool(name="accp", bufs=1, space="PSUM"))

        rsc_sb = singles.tile([P, T], dt.float32)
        nc.scalar.dma_start(out=rsc_sb[:], in_=rsc)
        g_sb = singles.tile([P, N], dt.float16)
        for eng, lo, hi in (
            (nc.sync, 0, 768),
            (nc.scalar, 768, 1536),
            (nc.gpsimd, 1536, 2048),
        ):
            eng.dma_start(out=g_sb[:, lo:hi], in_=g[:, lo:hi])
        wha_sb = singles.tile([P, T * 65], dt.float16)
        nc.scalar.dma_start(out=wha_sb[:], in_=wha)
        a16_sb = singles.tile([P, NTT * N], dt.float16, name="a16")
        nc.sync.dma_start(out=a16_sb[:, 0 : 2 * N], in_=adj16[:, 0 : 2 * N])
        nc.sync.dma_start(out=a16_sb[:, 2 * N : 4 * N], in_=adj16[:, 2 * N : 4 * N])

        accs = {}
        for ph in range(2):
            for n in range(NB):
                accs[ph, n] = accp.tile(
                    [65, 512], dt.float32, tag=f"acc{ph}_{n}", name=f"acc{ph}_{n}"
                )

        # PE clock-ramp: junk matmuls into the (not yet live) phase-B banks.
        for w in range(NWARM):
            nc.tensor.matmul(
                out=accs[1, w % NB][:],
                lhsT=wha_sb[:, 0:65],
                rhs=g_sb[:, 0:512],
                start=True,
                stop=True,
            )

        # Scores for the accum-DMA tiles (t = NTT..T-1), in GRP groups, then
        # their accumulating mask DMAs. Emitted first so the software-DGE
        # issue chain (the long pole) starts as early as possible.
        sgrp = {}
        for gi in range((T - NTT) // GRP):
            s = singles.tile([P, GRP * N], dt.float16, name=f"s{gi}")
            sgrp[gi] = s
            for k in range(GRP):
                t = NTT + gi * GRP + k
                nc.vector.tensor_scalar_max(
                    s[:, k * N : (k + 1) * N], g_sb[:], rsc_sb[:, t : t + 1]
                )
            for k in range(GRP):
                t = NTT + gi * GRP + k
                nc.gpsimd.dma_start(
                    out=s[:, k * N : (k + 1) * N],
                    in_=adjenc[:, (t - NTT) * N : (t - NTT + 1) * N],
                    accum_op=mybir.AluOpType.add,
                )

        osbA = singles.tile([65, N], dt.float16, name="osbA")
        osbB = singles.tile([65, N], dt.float16, name="osbB")

        def feed_pe(t, q_ap):
            ph, first, last = t // 8, t % 8 == 0, t % 8 == 7
            for n in range(NB):
                nc.tensor.matmul(
                    out=accs[ph, n][:],
                    lhsT=wha_sb[:, t * 65 : (t + 1) * 65],
                    rhs=q_ap[:, n * 512 : (n + 1) * 512],
                    start=first,
                    stop=last,
                )
            if last:
                osb, outd = (osbA, outA) if ph == 0 else (osbB, outB)
                for n in range(NB):
                    dst = osb[:, n * 512 : (n + 1) * 512]
                    if n % 2 == 0:
                        nc.scalar.copy(dst, accs[ph, n][:])
                    else:
                        nc.vector.tensor_copy(dst, accs[ph, n][:])
                nc.sync.dma_start(out=outd[:, 0 : N // 2], in_=osb[:, 0 : N // 2])
                nc.sync.dma_start(out=outd[:, N // 2 :], in_=osb[:, N // 2 :])

        # tensor_tensor-masked leading tiles (adj fp16 arrives fast).
        spool = ctx.enter_context(tc.tile_pool(name="spool", bufs=2))
        for t in range(NTT):
            st = spool.tile([P, N], dt.float16, name="st")
            nc.vector.tensor_scalar_max(st[:], g_sb[:], rsc_sb[:, t : t + 1])
            qt = qpool.tile([P, N], dt.float16, name="qt")
            nc.vector.tensor_tensor(
                qt[:], st[:], a16_sb[:, t * N : (t + 1) * N], mybir.AluOpType.mult
            )
            feed_pe(t, qt)

        # Merged relus over each accum group, then PE.
        for gi in range((T - NTT) // GRP):
            qg = qpool.tile([P, GRP * N], dt.float16, name="qg")
            nc.vector.tensor_scalar_max(qg[:], sgrp[gi][:], 0.0)
            for k in range(GRP):
                t = NTT + gi * GRP + k
                feed_pe(t, qg[:, k * N : (k + 1) * N])

    nc.compile()
    _CACHE["nc"] = nc
    return nc


def _prep_inputs(h, adj, W, a):
    h = np.asarray(h, np.float32)
    adj = np.asarray(adj, np.float32)
    W = np.asarray(W, np.float32)
    a = np.asarray(a, np.float32)

    adjT = adj.T.reshape(T, P, N)
    a16 = np.ascontiguousarray(
        adjT[:NTT].transpose(1, 0, 2).reshape(P, NTT * N)
    ).astype(np.float16)
    adjenc = np.ascontiguousarray(
        ((adjT[NTT:] - 1.0) * 448.0).transpose(1, 0, 2).reshape(P, (T - NTT) * N)
    ).astype(ml_dtypes.float8_e4m3)

    Wh = np.einsum("bnf,of->bno", h, W)  # [B, N, F]
    e1 = Wh @ a[:F]  # [B, N]
    e2 = Wh @ a[F:]  # [B, N]
    A2 = np.exp(e2)
    G = np.exp(0.8 * e1).astype(np.float16)  # [B, N]
    r = np.exp(-0.8 * e2).astype(np.float32)  # [B, N]
    whA = np.concatenate([Wh * A2[..., None], A2[..., None]], axis=2)  # [B, N, 65]
    whA = np.ascontiguousarray(
        whA.reshape(B, T, P, 65).transpose(0, 2, 1, 3)
    ).reshape(B, P, T * 65)

    in_maps = []
    for b in range(B):
        in_maps.append(
            {
                "adj16": a16,
                "adjenc": adjenc,
                "g": np.ascontiguousarray(np.broadcast_to(G[b], (P, N))),
                "rsc": np.ascontiguousarray(r[b].reshape(T, P).T),
                "wha": whA[b].astype(np.float16),
            }
        )
    return in_maps


def kernel(h, adj, W, a, _trace=False):
    nc = _build_program()
    in_maps = _prep_inputs(h, adj, W, a)
    res = run_bass_kernel_spmd(nc, in_maps, list(range(B)), trace=_trace)
    outs = np.empty((B, N, F), np.float32)
    for b in range(B):
        outT = np.asarray(res.results[b]["outA"], np.float32) + np.asarray(
            res.results[b]["outB"], np.float32
        )
        hp = outT[:F].T / outT[F][:, None]
        outs[b] = np.where(hp > 0, hp, np.expm1(hp))
    if _trace:
        kernel.last_results = res
    return outs


# revision 23
# speedup vs baseline: 1.1125x; 1.1089x over previous
"""GAT layer (B=8, N=2048, F=64) on 8 trn2 NeuronCores.

Strategy: data-parallel over batch B — one graph per core, adj replicated.

Math: with e = leaky_relu(e1_i + e2_j), exp(e - 0.2*e1_i) (row factor
cancels in softmax) = A2_j * max(G_i, r_j) where G = exp(0.8*e1),
A2 = exp(e2), r = exp(-0.8*e2). A2 folds into the matmul weights on the
host (whA = [Wh*A2 | A2]; row 64 yields softmax denominators), so the
device computes q_ij = max(G_i, r_j) * adj_ji and accumulates
outT[65, i] += whA_t^T @ q_t over 16 j-tiles. Divide + elu epilogue is
O(N*F) and runs on the host.

The N^2 elementwise stage runs entirely on DVE, whose fast modes need
all-2-byte SBUF operands (hence adj in fp16):
    s_t = (G max r_t)   tensor_scalar, 4x mode, ~0.65us/tile
    q_t = s_t * adj_t   tensor_tensor, 2x mode, ~1.2us/tile
~30us total; measured alternatives all lose: fused scalar_tensor_tensor
runs 1x (~37us), GpSimd/Pool tensor ops poison DVE's fast modes
(2.5-6x degradation), and gpsimd accumulating-DMA masking is issue- and
transfer-bound (~2.5-5us/tile).

Startup/tail, all measured pressure points:
  - DMA bandwidth is shared round-robin across in-flight DMAs, so G
    (host-replicated; a stride-0 broadcast DMA is ~5x slower) issues
    FIRST in 2 chunks on the 2 hwdge queues, before the 8.4MB of adj
    competes; scores only need G and front-run the adj fill.
  - adj arrives as 16 per-tile DMAs -> mask t unblocks on its own tile.
  - A few junk matmuls ramp PE's clock (2.4GHz needs ~3us busy) during
    the fill so the accumulation chain tracks the DVE stream.
  - The 16-tile accumulation runs as two 4-bank PSUM phases; phase-A
    copies (split ACT/DVE) + halved output DMAs overlap phase B; the
    host adds the halves.
"""

import sys

import numpy as np

for _p in ("/opt/trn_rl_repo",):
    if _p not in sys.path:
        sys.path.insert(0, _p)

from contextlib import ExitStack

import concourse.bass as bass
import concourse.tile as tile
from concourse import bacc, mybir
from concourse.bass_utils import run_bass_kernel_spmd

B, N, F = 8, 2048, 64
P = 128
T = N // P  # 16 j-tiles
NB = N // 512  # 4 psum banks of moving-free 512
PREFETCH = 4  # scores emitted ahead of the mask loop
NWARM = 4  # PE clock-ramp matmuls during the DMA fill

_CACHE = {}


def _build_program():
    if "nc" in _CACHE:
        return _CACHE["nc"]
    dt = mybir.dt
    nc = bacc.Bacc("TRN2", target_bir_lowering=False, debug=False)

    adjd = nc.dram_tensor("adjd", [P, T * N], dt.float16, kind="ExternalInput").ap()
    g = nc.dram_tensor("g", [P, N], dt.float16, kind="ExternalInput").ap()
    rsc = nc.dram_tensor("rsc", [P, T], dt.float32, kind="ExternalInput").ap()
    wha = nc.dram_tensor("wha", [P, T * 65], dt.float16, kind="ExternalInput").ap()
    outA = nc.dram_tensor("outA", [65, N], dt.float16, kind="ExternalOutput").ap()
    outB = nc.dram_tensor("outB", [65, N], dt.float16, kind="ExternalOutput").ap()

    with tile.TileContext(nc) as tc, ExitStack() as ctx:
        singles = ctx.enter_context(tc.tile_pool(name="singles", bufs=1))
        spool = ctx.enter_context(tc.tile_pool(name="spool", bufs=PREFETCH + 4))
        qpool = ctx.enter_context(tc.tile_pool(name="qpool", bufs=6))
        accp = ctx.enter_context(tc.tile_pool(name="accp", bufs=1, space="PSUM"))

        # G first: its 2 chunks get the DMA fabric to themselves briefly.
        g_sb = singles.tile([P, N], dt.float16)
        nc.sync.dma_start(out=g_sb[:, 0:1024], in_=g[:, 0:1024])
        nc.scalar.dma_start(out=g_sb[:, 1024:2048], in_=g[:, 1024:2048])
        rsc_sb = singles.tile([P, T], dt.float32)
        nc.scalar.dma_start(out=rsc_sb[:], in_=rsc)
        wha_sb = singles.tile([P, T * 65], dt.float16)
        nc.scalar.dma_start(out=wha_sb[:], in_=wha)

        adj_sb = singles.tile([P, T * N], dt.float16, name="adj")
        for t in range(T):
            nc.sync.dma_start(
                out=adj_sb[:, t * N : (t + 1) * N], in_=adjd[:, t * N : (t + 1) * N]
            )

        def make_score(t):
            st = spool.tile([P, N], dt.float16, name="s")
            nc.vector.tensor_scalar_max(st[:], g_sb[:], rsc_sb[:, t : t + 1])
            return st

        scores = {t: make_score(t) for t in range(PREFETCH)}

        accs = {}
        for ph in range(2):
            for n in range(NB):
                accs[ph, n] = accp.tile(
                    [65, 512], dt.float32, tag=f"acc{ph}_{n}", name=f"acc{ph}_{n}"
                )

        # PE clock-ramp: junk matmuls into the (not yet live) phase-B banks,
        # reading only g_sb (available early).
        for w in range(NWARM):
            nc.tensor.matmul(
                out=accs[1, w % NB][:],
                lhsT=g_sb[:, 0:65],
                rhs=g_sb[:, 0:512],
                start=True,
                stop=True,
            )

        osbA = singles.tile([65, N], dt.float16, name="osbA")
        osbB = singles.tile([65, N], dt.float16, name="osbB")

        for t in range(T):
            ph, first, last = t // 8, t % 8 == 0, t % 8 == 7
            st = scores.pop(t) if t in scores else make_score(t)
            qt = qpool.tile([P, N], dt.float16)
            nc.vector.tensor_tensor(
                qt[:], st[:], adj_sb[:, t * N : (t + 1) * N], mybir.AluOpType.mult
            )
            for n in range(NB):
                nc.tensor.matmul(
                    out=accs[ph, n][:],
                    lhsT=wha_sb[:, t * 65 : (t + 1) * 65],
                    rhs=qt[:, n * 512 : (n + 1) * 512],
                    start=first,
                    stop=last,
                )
            if last:
                osb, outd = (osbA, outA) if ph == 0 else (osbB, outB)
                for n in range(NB):
                    dst = osb[:, n * 512 : (n + 1) * 512]
                    if n % 2 == 0:
                        nc.scalar.copy(dst, accs[ph, n][:])
                    else:
                        nc.vector.tensor_copy(dst, accs[ph, n][:])
                nc.sync.dma_start(out=outd[:, 0 : N // 2], in_=osb[:, 0 : N // 2])
                nc.sync.dma_start(out=outd[:, N // 2 :], in_=osb[:, N // 2 :])

    nc.compile()
    _CACHE["nc"] = nc
    return nc


def _prep_inputs(h, adj, W, a):
    h = np.asarray(h, np.float32)
    adj = np.asarray(adj, np.float32)
    W = np.asarray(W, np.float32)
    a = np.asarray(a, np.float32)

    # adj^T tiles side by side along free dim: adjd[p, t*N + i] = adjT[t*128+p, i]
    adjd = np.ascontiguousarray(
        adj.T.reshape(T, P, N).transpose(1, 0, 2).reshape(P, T * N)
    ).astype(np.float16)

    Wh = np.einsum("bnf,of->bno", h, W)  # [B, N, F]
    e1 = Wh @ a[:F]  # [B, N]
    e2 = Wh @ a[F:]  # [B, N]
    A2 = np.exp(e2)
    G = np.exp(0.8 * e1).astype(np.float16)  # [B, N]
    r = np.exp(-0.8 * e2).astype(np.float32)  # [B, N]
    whA = np.concatenate([Wh * A2[..., None], A2[..., None]], axis=2)  # [B, N, 65]
    whA = np.ascontiguousarray(
        whA.reshape(B, T, P, 65).transpose(0, 2, 1, 3)
    ).reshape(B, P, T * 65)

    in_maps = []
    for b in range(B):
        in_maps.append(
            {
                "adjd": adjd,
                "g": np.ascontiguousarray(np.broadcast_to(G[b], (P, N))),
                "rsc": np.ascontiguousarray(r[b].reshape(T, P).T),
                "wha": whA[b].astype(np.float16),
            }
        )
    return in_maps


def kernel(h, adj, W, a, _trace=False):
    nc = _build_program()
    in_maps = _prep_inputs(h, adj, W, a)
    res = run_bass_kernel_spmd(nc, in_maps, list(range(B)), trace=_trace)
    outs = np.empty((B, N, F), np.float32)
    for b in range(B):
        outT = np.asarray(res.results[b]["outA"], np.float32) + np.asarray(
            res.results[b]["outB"], np.float32
        )
        hp = outT[:F].T / outT[F][:, None]
        outs[b] = np.where(hp > 0, hp, np.expm1(hp))
    if _trace:
        kernel.last_results = res
    return outs
